# revision 10
# baseline (speedup 1.0000x reference)
"""STFT (DFT-as-conv) kernel for Trainium2, 8 NeuronCores.

Problem: x (16, 262144) f32, hann-windowed DFT kernels wsin/wcos
(2048, 1, 2048); reference reflect-pads by 1024, convolves with hop 512
-> returns (real, -imag), each (16, 2048, 513) f32.

v3 strategy (v1 was DMA-bound at 47us f32; v2 all-bf16 hit DVE
queue head-of-line stalls from on-device folds):
  - Same folded math as v1: time-reversal fold halves contraction to
    1024 lanes (n = 256c + 2jj + par, c = 0..3); bin-parity fold gives
    bins k and 1024-k as E +/- O from even/odd lanes; host assembles
    mirrors; bin 512 is a host matvec.
  - Folds on HOST; device only matmuls + PSUM->SBUF copies + DMA.
  - Mixed precision by window magnitude: hann window makes lanes
    c in {0,1} (win < 0.5) carry ~8% of the quantization-error
    variance, so they ship as fp8e4 and contract in one DoubleRow
    matmul (0.5 cycles/col for 256 lanes); c in {2,3} stay bf16
    (1 cycle/col each). Measured max-norm rel err 0.84e-2 (gate 2e-2;
    all-fp8 was 3e-2 and all-bf16 2.9e-3 at 1.6x the cycles).
  - PE cost: 2.5 cols/chain-group vs v1's 4; DMA 8.6 MB/core vs 17.
  - Outputs ship bf16 per (up, b) on the Pool SWDGE queue so output
    dispatch never blocks input DMAs on HWDGE.
"""

import sys

sys.path.insert(0, "/opt/trn_rl_repo")

import numpy as np
import ml_dtypes

BATCH = 16
LENGTH = 262144
N_FFT = 2048
HOP = 512
FRAMES = 513          # LENGTH // HOP + 1
N_GROUPS = ((0, 257), (257, 256))  # frame groups (PSUM bank caps N at 512)
CORES = 8
B_PER_CORE = BATCH // CORES
N_UP = 8              # u' = kern*4 + mc, bins 0..511 in 4 chunks per kern
ZB_SPLIT = 264        # first-frame-half split of the first bf16 z DMA

_cache = {}


def _build_device_kernel():
    import concourse.bacc as bacc
    import concourse.mybir as mybir
    from concourse import tile

    nc = bacc.Bacc("TRN2", target_bir_lowering=False, debug=False,
                   num_devices=CORES)
    f32 = mybir.dt.float32
    bf16 = mybir.dt.bfloat16
    f8 = mybir.dt.float8e4
    DR = mybir.MatmulPerfMode.DoubleRow

    # z8[b, s, p, par, c, f]: folded frames, lanes c in {0,1} (fp8);
    #   lane n = 256c + 2p + par holds y_f[n] +/- y_f[2048-n] (s: 0=+,1=-)
    z8_d = nc.dram_tensor("z8", [B_PER_CORE, 2, 128, 2, 2, FRAMES], f8,
                          kind="ExternalInput")
    # zb[b, s, p, par, c, f]: lanes c in {2,3} (bf16)
    zb_d = nc.dram_tensor("zb", [B_PER_CORE, 2, 128, 2, 2, FRAMES], bf16,
                          kind="ExternalInput")
    # w8[p, up, par, t, k]: folded parity weights, lanes c = t in {0,1}
    w8_d = nc.dram_tensor("w8", [128, N_UP, 2, 2, 128], f8,
                          kind="ExternalInput")
    # wb[p, up, par, c, k]: lanes c in {2,3}
    wb_d = nc.dram_tensor("wb", [128, N_UP, 2, 2, 128], bf16,
                          kind="ExternalInput")
    # o[up, k, b*2F + half*F + f]: half 0 = E, 1 = O
    o_d = nc.dram_tensor("o", [N_UP, 128, B_PER_CORE * 2 * FRAMES],
                         bf16, kind="ExternalOutput")

    with tile.TileContext(nc) as tc:
        with (
            tc.tile_pool(name="zp", bufs=1) as zp,
            tc.tile_pool(name="wp", bufs=1) as wp,
            tc.tile_pool(name="op", bufs=8) as op,
            tc.tile_pool(name="psp", bufs=4, space="PSUM") as psp,
        ):
            z8t = [[None] * 2 for _ in range(B_PER_CORE)]
            zbt = [[None] * 2 for _ in range(B_PER_CORE)]
            for b in range(B_PER_CORE):
                for s in range(2):
                    z8t[b][s] = zp.tile([128, 2, 2, FRAMES], f8,
                                        name=f"z8{b}{s}", tag=f"z8{b}{s}")
                    zbt[b][s] = zp.tile([128, 2, 2, FRAMES], bf16,
                                        name=f"zb{b}{s}", tag=f"zb{b}{s}")
            w8t = wp.tile([128, N_UP, 2, 2, 128], f8, name="w8t", tag="w8t")
            wbt = wp.tile([128, N_UP, 2, 2, 128], bf16, name="wbt",
                          tag="wbt")
            ots = [op.tile([128, B_PER_CORE * 2 * FRAMES], bf16,
                           name=f"ot{u}", tag="ot") for u in range(N_UP)]

            in_q = nc.sync
            w_q = nc.scalar
            out_q = nc.sync

            # DMA schedule in order of first use. Weights ride the scalar
            # queue so their HWDGE dispatch overlaps the z dispatches on
            # sync; arrival order at the (serial) DMA engines stays packed.
            w_q.dma_start(out=w8t[:, 0], in_=w8_d[:, 0])
            in_q.dma_start(out=z8t[0][0], in_=z8_d[0, 0])
            w_q.dma_start(out=wbt[:, 0], in_=wb_d[:, 0])
            in_q.dma_start(out=zbt[0][0][:, :, :, :ZB_SPLIT],
                           in_=zb_d[0, 0, :, :, :, :ZB_SPLIT])
            in_q.dma_start(out=zbt[0][0][:, :, :, ZB_SPLIT:],
                           in_=zb_d[0, 0, :, :, :, ZB_SPLIT:])
            in_q.dma_start(out=z8t[1][0], in_=z8_d[1, 0])
            in_q.dma_start(out=zbt[1][0], in_=zb_d[1, 0])
            w_q.dma_start(out=w8t[:, 1:], in_=w8_d[:, 1:])
            w_q.dma_start(out=wbt[:, 1:4], in_=wb_d[:, 1:4])
            w_q.dma_start(out=wbt[:, 4:], in_=wb_d[:, 4:])
            for b in range(B_PER_CORE):
                in_q.dma_start(out=z8t[b][1], in_=z8_d[b, 1])
                in_q.dma_start(out=zbt[b][1], in_=zb_d[b, 1])

            # s=0 phase alternates batches (b1 data lands early); s=1 phase
            # runs b=0 units first so the late-arriving b=1 s=1 tiles are
            # only needed at the very end.
            sched = ([(up, b) for up in range(4) for b in range(B_PER_CORE)]
                     + [(up, 0) for up in range(4, N_UP)]
                     + [(up, 1) for up in range(4, N_UP)])
            copy_eng = (nc.vector.tensor_copy, nc.scalar.copy)
            ci = 0
            for up, b in sched:
                kern = up // 4
                ot = ots[up]
                base = b * 2 * FRAMES
                for f0, ng in N_GROUPS:
                    psE = psp.tile([128, ng], f32, name="psE", tag="psE")
                    psO = psp.tile([128, ng], f32, name="psO", tag="psO")
                    for par, ps in ((0, psE), (1, psO)):
                        nc.tensor.matmul(
                            ps, w8t[:, up, par],
                            z8t[b][kern][:, par, :, f0:f0 + ng],
                            start=True, stop=False, perf_mode=DR,
                            skip_group_check=True)
                        for c in range(2):
                            nc.tensor.matmul(
                                ps, wbt[:, up, par, c],
                                zbt[b][kern][:, par, c, f0:f0 + ng],
                                start=False, stop=(c == 1),
                                skip_group_check=True)
                    copy_eng[ci % 2](out=ot[:, base + f0:base + f0 + ng],
                                     in_=psE)
                    copy_eng[(ci + 1) % 2](
                        out=ot[:, base + FRAMES + f0:base + FRAMES + f0 + ng],
                        in_=psO)
                    ci += 1
                if (up, b) in sched[-2:]:
                    # split the final output DMA so its first half overlaps
                    # the last copies instead of bunching at the end
                    out_q.dma_start(out=o_d[up, :, base:base + FRAMES],
                                    in_=ot[:, base:base + FRAMES])
                    out_q.dma_start(
                        out=o_d[up, :, base + FRAMES:base + 2 * FRAMES],
                        in_=ot[:, base + FRAMES:base + 2 * FRAMES])
                else:
                    out_q.dma_start(
                        out=o_d[up, :, base:base + 2 * FRAMES],
                        in_=ot[:, base:base + 2 * FRAMES])
    nc.compile()
    return nc


def _get_nc():
    if "nc" not in _cache:
        _cache["nc"] = _build_device_kernel()
    return _cache["nc"]


def _host_prep(x, wsin, wcos):
    bf16 = ml_dtypes.bfloat16
    f8 = ml_dtypes.float8_e4m3
    x = np.asarray(x, dtype=np.float32)
    wsin = np.asarray(wsin, dtype=np.float32).reshape(N_FFT, N_FFT)
    wcos = np.asarray(wcos, dtype=np.float32).reshape(N_FFT, N_FFT)

    xpad = np.pad(x, ((0, 0), (N_FFT // 2, N_FFT // 2)), mode="reflect")
    npad = xpad.shape[1]

    # folded frames z[par][s][b, c, p, f]; lane n = 256c + 2p + par
    f_idx = np.arange(FRAMES)
    p_idx = np.arange(128)
    c_idx = np.arange(4)
    n_lane = 256 * c_idx[:, None] + 2 * p_idx[None, :]
    z8 = np.empty((B_PER_CORE * CORES // B_PER_CORE, 0))  # placeholder
    z8 = np.empty((BATCH, 2, 128, 2, 2, FRAMES), f8)
    zb = np.empty((BATCH, 2, 128, 2, 2, FRAMES), bf16)
    for par in range(2):
        n = n_lane + par
        idx_f = 512 * f_idx[None, None, :] + n[:, :, None]
        idx_r = np.clip(512 * f_idx[None, None, :] + (2048 - n)[:, :, None],
                        0, npad - 1)
        y = xpad[:, idx_f]                       # (B, 4, 128, F)
        yr = xpad[:, idx_r].copy()
        if par == 0:
            yr[:, 0, 0, :] = 0.0                 # n=0 lane unpaired
        zp = y + yr
        zm = y - yr
        if par == 0:
            # n=0 even lane is dead (win[0] = 0): its weight slot carries
            # the cos n=1024 column, so the lane must hold y_f[1024].
            idx1024 = np.clip(512 * f_idx + 1024, 0, npad - 1)
            zp[:, 0, 0, :] = xpad[:, idx1024]
        for s, arr in ((0, zp), (1, zm)):
            z8[:, s, :, par, :, :] = arr[:, 0:2].transpose(0, 2, 1, 3)
            zb[:, s, :, par, :, :] = arr[:, 2:4].transpose(0, 2, 1, 3)

    # folded parity weights for bin rows k < 512: wf[up, p, par, c, k]
    wf = np.empty((8, 128, 2, 4, 128), np.float32)
    for kern, wm in enumerate((wcos, -wsin)):
        for mc in range(4):
            rows = wm[128 * mc:128 * mc + 128]       # (128 bins, 2048)
            for c in range(4):
                n_ev = 256 * c + 2 * p_idx
                wf[kern * 4 + mc, :, 0, c, :] = rows[:, n_ev].T
                wf[kern * 4 + mc, :, 1, c, :] = rows[:, n_ev + 1].T
    # n=0 even lane is dead (win[0] = 0): carry the cos n=1024 column
    wf[0:4, 0, 0, 0, :] = wcos[:512, 1024].reshape(4, 128)
    w8 = np.ascontiguousarray(
        wf[:, :, :, 0:2, :].transpose(1, 0, 2, 3, 4)).astype(f8)
    wb = np.ascontiguousarray(
        wf[:, :, :, 2:4, :].transpose(1, 0, 2, 3, 4)).astype(bf16)

    # host bin-512 rows (not representable in the parity fold)
    fr = np.lib.stride_tricks.sliding_window_view(
        xpad, N_FFT, axis=1)[:, ::HOP]               # (B, 513, 2048)
    row512 = np.empty((2, BATCH, FRAMES), np.float32)
    for kern, wm in enumerate((wcos, -wsin)):
        row512[kern] = np.einsum('bfn,n->bf', fr, wm[512],
                                 optimize=True).astype(np.float32)
    return z8, zb, w8, wb, row512


def _host_assemble(outs, row512):
    # outs: 8 arrays (8, 128, 2*2*513) bf16; E/O halves per batch
    per_batch_E, per_batch_O = [], []
    for o in outs:
        o = np.asarray(o, dtype=np.float32)
        for b in range(B_PER_CORE):
            base = b * 2 * FRAMES
            per_batch_E.append(o[:, :, base:base + FRAMES])
            per_batch_O.append(
                o[:, :, base + FRAMES:base + 2 * FRAMES])
    E = np.stack(per_batch_E).reshape(BATCH, 2, 512, FRAMES)
    O = np.stack(per_batch_O).reshape(BATCH, 2, 512, FRAMES)

    outs_full = []
    for kern, msign in ((0, 1.0), (1, -1.0)):
        lo = E[:, kern] + O[:, kern]               # bins 0..511
        hi = E[:, kern] - O[:, kern]               # bins 1024-k
        if kern == 1:
            hi = -hi
        head = np.concatenate(
            [lo, row512[kern][:, None, :], hi[:, 511:0:-1], hi[:, 0:1]],
            axis=1)                                 # bins 0..1024
        full = np.concatenate([head, msign * head[:, 1023:0:-1]], axis=1)
        outs_full.append(np.ascontiguousarray(full, dtype=np.float32))
    return tuple(outs_full)


def kernel(x, wsin, wcos):
    from concourse.bass_utils import run_bass_kernel_spmd

    nc = _get_nc()
    z8, zb, w8, wb, row512 = _host_prep(x, wsin, wcos)
    in_maps = [
        {"z8": z8[i * B_PER_CORE:(i + 1) * B_PER_CORE],
         "zb": zb[i * B_PER_CORE:(i + 1) * B_PER_CORE],
         "w8": w8, "wb": wb}
        for i in range(CORES)
    ]
    res = run_bass_kernel_spmd(nc, in_maps, core_ids=list(range(CORES)))
    return _host_assemble(
        [res.results[i]["o"] for i in range(CORES)], row512)


# revision 12
# speedup vs baseline: 1.0126x; 1.0126x over previous
"""STFT (DFT-as-conv) kernel for Trainium2, 8 NeuronCores.

Problem: x (16, 262144) f32, hann-windowed DFT kernels wsin/wcos
(2048, 1, 2048); reference reflect-pads by 1024, convolves with hop 512
-> returns (real, -imag), each (16, 2048, 513) f32.

v3 strategy (v1 was DMA-bound at 47us f32; v2 all-bf16 hit DVE
queue head-of-line stalls from on-device folds):
  - Same folded math as v1: time-reversal fold halves contraction to
    1024 lanes (n = 256c + 2jj + par, c = 0..3); bin-parity fold gives
    bins k and 1024-k as E +/- O from even/odd lanes; host assembles
    mirrors; bin 512 is a host matvec.
  - Folds on HOST; device only matmuls + PSUM->SBUF copies + DMA.
  - Mixed precision by window magnitude: hann window makes lanes
    c in {0,1} (win < 0.5) carry ~8% of the quantization-error
    variance, so they ship as fp8e4 and contract in one DoubleRow
    matmul (0.5 cycles/col for 256 lanes); c in {2,3} stay bf16
    (1 cycle/col each). Measured max-norm rel err 0.84e-2 (gate 2e-2;
    all-fp8 was 3e-2 and all-bf16 2.9e-3 at 1.6x the cycles).
  - PE cost: 2.5 cols/chain-group vs v1's 4; DMA 8.6 MB/core vs 17.
  - Outputs ship bf16 per (up, b) on the Pool SWDGE queue so output
    dispatch never blocks input DMAs on HWDGE.
"""

import sys

sys.path.insert(0, "/opt/trn_rl_repo")

import numpy as np
import ml_dtypes

BATCH = 16
LENGTH = 262144
N_FFT = 2048
HOP = 512
FRAMES = 513          # LENGTH // HOP + 1
N_GROUPS = ((0, 257), (257, 256))  # frame groups (PSUM bank caps N at 512)
CORES = 8
B_PER_CORE = BATCH // CORES
N_UP = 8              # u' = kern*4 + mc, bins 0..511 in 4 chunks per kern
ZB_SPLIT = 264        # first-frame-half split of the first bf16 z DMA

_cache = {}


def _build_device_kernel():
    import concourse.bacc as bacc
    import concourse.mybir as mybir
    from concourse import tile

    nc = bacc.Bacc("TRN2", target_bir_lowering=False, debug=False,
                   num_devices=CORES)
    f32 = mybir.dt.float32
    bf16 = mybir.dt.bfloat16
    f8 = mybir.dt.float8e4
    DR = mybir.MatmulPerfMode.DoubleRow

    # z8[b, s, p, par, c, f]: folded frames, lanes c in {0,1} (fp8);
    #   lane n = 256c + 2p + par holds y_f[n] +/- y_f[2048-n] (s: 0=+,1=-)
    z8_d = nc.dram_tensor("z8", [B_PER_CORE, 2, 128, 2, 2, FRAMES], f8,
                          kind="ExternalInput")
    # zb[b, s, p, par, c, f]: lanes c in {2,3} (bf16)
    zb_d = nc.dram_tensor("zb", [B_PER_CORE, 2, 128, 2, 2, FRAMES], bf16,
                          kind="ExternalInput")
    # w8[p, up, par, t, k]: folded parity weights, lanes c = t in {0,1}
    w8_d = nc.dram_tensor("w8", [128, N_UP, 2, 2, 128], f8,
                          kind="ExternalInput")
    # wb[p, up, par, c, k]: lanes c in {2,3}
    wb_d = nc.dram_tensor("wb", [128, N_UP, 2, 2, 128], bf16,
                          kind="ExternalInput")
    # o[up, k, b*2F + half*F + f]: half 0 = E, 1 = O
    o_d = nc.dram_tensor("o", [N_UP, 128, B_PER_CORE * 2 * FRAMES],
                         bf16, kind="ExternalOutput")

    with tile.TileContext(nc) as tc:
        with (
            tc.tile_pool(name="zp", bufs=1) as zp,
            tc.tile_pool(name="wp", bufs=1) as wp,
            tc.tile_pool(name="op", bufs=8) as op,
            tc.tile_pool(name="psp", bufs=4, space="PSUM") as psp,
        ):
            z8t = [[None] * 2 for _ in range(B_PER_CORE)]
            zbt = [[None] * 2 for _ in range(B_PER_CORE)]
            for b in range(B_PER_CORE):
                for s in range(2):
                    z8t[b][s] = zp.tile([128, 2, 2, FRAMES], f8,
                                        name=f"z8{b}{s}", tag=f"z8{b}{s}")
                    zbt[b][s] = zp.tile([128, 2, 2, FRAMES], bf16,
                                        name=f"zb{b}{s}", tag=f"zb{b}{s}")
            w8t = wp.tile([128, N_UP, 2, 2, 128], f8, name="w8t", tag="w8t")
            wbt = wp.tile([128, N_UP, 2, 2, 128], bf16, name="wbt",
                          tag="wbt")
            ots = [op.tile([128, B_PER_CORE * 2 * FRAMES], bf16,
                           name=f"ot{u}", tag="ot") for u in range(N_UP)]

            in_q = nc.sync
            w_q = nc.scalar
            out_qs = (nc.sync, nc.gpsimd)  # alternate so dispatches overlap

            # DMA schedule in order of first use. Weights ride the scalar
            # queue so their HWDGE dispatch overlaps the z dispatches on
            # sync; arrival order at the (serial) DMA engines stays packed.
            w_q.dma_start(out=w8t[:, 0], in_=w8_d[:, 0])
            in_q.dma_start(out=z8t[0][0], in_=z8_d[0, 0])
            w_q.dma_start(out=wbt[:, 0], in_=wb_d[:, 0])
            in_q.dma_start(out=zbt[0][0][:, :, :, :ZB_SPLIT],
                           in_=zb_d[0, 0, :, :, :, :ZB_SPLIT])
            in_q.dma_start(out=zbt[0][0][:, :, :, ZB_SPLIT:],
                           in_=zb_d[0, 0, :, :, :, ZB_SPLIT:])
            in_q.dma_start(out=z8t[1][0], in_=z8_d[1, 0])
            in_q.dma_start(out=zbt[1][0], in_=zb_d[1, 0])
            w_q.dma_start(out=w8t[:, 1:], in_=w8_d[:, 1:])
            w_q.dma_start(out=wbt[:, 1:4], in_=wb_d[:, 1:4])
            w_q.dma_start(out=wbt[:, 4:], in_=wb_d[:, 4:])
            for b in range(B_PER_CORE):
                in_q.dma_start(out=z8t[b][1], in_=z8_d[b, 1])
                in_q.dma_start(out=zbt[b][1], in_=zb_d[b, 1])

            # s=0 phase alternates batches (b1 data lands early); s=1 phase
            # runs b=0 units first so the late-arriving b=1 s=1 tiles are
            # only needed at the very end.
            sched = ([(up, b) for up in range(4) for b in range(B_PER_CORE)]
                     + [(up, 0) for up in range(4, N_UP)]
                     + [(up, 1) for up in range(4, N_UP)])
            copy_eng = (nc.vector.tensor_copy, nc.scalar.copy)
            ci = 0
            for up, b in sched:
                kern = up // 4
                ot = ots[up]
                base = b * 2 * FRAMES
                for f0, ng in N_GROUPS:
                    psE = psp.tile([128, ng], f32, name="psE", tag="psE")
                    psO = psp.tile([128, ng], f32, name="psO", tag="psO")
                    for par, ps in ((0, psE), (1, psO)):
                        nc.tensor.matmul(
                            ps, w8t[:, up, par],
                            z8t[b][kern][:, par, :, f0:f0 + ng],
                            start=True, stop=False, perf_mode=DR,
                            skip_group_check=True)
                        for c in range(2):
                            nc.tensor.matmul(
                                ps, wbt[:, up, par, c],
                                zbt[b][kern][:, par, c, f0:f0 + ng],
                                start=False, stop=(c == 1),
                                skip_group_check=True)
                    copy_eng[ci % 2](out=ot[:, base + f0:base + f0 + ng],
                                     in_=psE)
                    copy_eng[(ci + 1) % 2](
                        out=ot[:, base + FRAMES + f0:base + FRAMES + f0 + ng],
                        in_=psO)
                    ci += 1
                ui = sched.index((up, b))
                if (up, b) in sched[-2:]:
                    # split the final output DMAs so their first halves
                    # overlap the last copies instead of bunching at the end
                    out_qs[ui % 2].dma_start(
                        out=o_d[up, :, base:base + FRAMES],
                        in_=ot[:, base:base + FRAMES])
                    out_qs[(ui + 1) % 2].dma_start(
                        out=o_d[up, :, base + FRAMES:base + 2 * FRAMES],
                        in_=ot[:, base + FRAMES:base + 2 * FRAMES])
                else:
                    out_qs[ui % 2].dma_start(
                        out=o_d[up, :, base:base + 2 * FRAMES],
                        in_=ot[:, base:base + 2 * FRAMES])
    nc.compile()
    return nc


def _get_nc():
    if "nc" not in _cache:
        _cache["nc"] = _build_device_kernel()
    return _cache["nc"]


def _host_prep(x, wsin, wcos):
    bf16 = ml_dtypes.bfloat16
    f8 = ml_dtypes.float8_e4m3
    x = np.asarray(x, dtype=np.float32)
    wsin = np.asarray(wsin, dtype=np.float32).reshape(N_FFT, N_FFT)
    wcos = np.asarray(wcos, dtype=np.float32).reshape(N_FFT, N_FFT)

    xpad = np.pad(x, ((0, 0), (N_FFT // 2, N_FFT // 2)), mode="reflect")
    npad = xpad.shape[1]

    # folded frames z[par][s][b, c, p, f]; lane n = 256c + 2p + par
    f_idx = np.arange(FRAMES)
    p_idx = np.arange(128)
    c_idx = np.arange(4)
    n_lane = 256 * c_idx[:, None] + 2 * p_idx[None, :]
    z8 = np.empty((B_PER_CORE * CORES // B_PER_CORE, 0))  # placeholder
    z8 = np.empty((BATCH, 2, 128, 2, 2, FRAMES), f8)
    zb = np.empty((BATCH, 2, 128, 2, 2, FRAMES), bf16)
    for par in range(2):
        n = n_lane + par
        idx_f = 512 * f_idx[None, None, :] + n[:, :, None]
        idx_r = np.clip(512 * f_idx[None, None, :] + (2048 - n)[:, :, None],
                        0, npad - 1)
        y = xpad[:, idx_f]                       # (B, 4, 128, F)
        yr = xpad[:, idx_r].copy()
        if par == 0:
            yr[:, 0, 0, :] = 0.0                 # n=0 lane unpaired
        zp = y + yr
        zm = y - yr
        if par == 0:
            # n=0 even lane is dead (win[0] = 0): its weight slot carries
            # the cos n=1024 column, so the lane must hold y_f[1024].
            idx1024 = np.clip(512 * f_idx + 1024, 0, npad - 1)
            zp[:, 0, 0, :] = xpad[:, idx1024]
        for s, arr in ((0, zp), (1, zm)):
            z8[:, s, :, par, :, :] = arr[:, 0:2].transpose(0, 2, 1, 3)
            zb[:, s, :, par, :, :] = arr[:, 2:4].transpose(0, 2, 1, 3)

    # folded parity weights for bin rows k < 512: wf[up, p, par, c, k]
    wf = np.empty((8, 128, 2, 4, 128), np.float32)
    for kern, wm in enumerate((wcos, -wsin)):
        for mc in range(4):
            rows = wm[128 * mc:128 * mc + 128]       # (128 bins, 2048)
            for c in range(4):
                n_ev = 256 * c + 2 * p_idx
                wf[kern * 4 + mc, :, 0, c, :] = rows[:, n_ev].T
                wf[kern * 4 + mc, :, 1, c, :] = rows[:, n_ev + 1].T
    # n=0 even lane is dead (win[0] = 0): carry the cos n=1024 column
    wf[0:4, 0, 0, 0, :] = wcos[:512, 1024].reshape(4, 128)
    w8 = np.ascontiguousarray(
        wf[:, :, :, 0:2, :].transpose(1, 0, 2, 3, 4)).astype(f8)
    wb = np.ascontiguousarray(
        wf[:, :, :, 2:4, :].transpose(1, 0, 2, 3, 4)).astype(bf16)

    # host bin-512 rows (not representable in the parity fold)
    fr = np.lib.stride_tricks.sliding_window_view(
        xpad, N_FFT, axis=1)[:, ::HOP]               # (B, 513, 2048)
    row512 = np.empty((2, BATCH, FRAMES), np.float32)
    for kern, wm in enumerate((wcos, -wsin)):
        row512[kern] = np.einsum('bfn,n->bf', fr, wm[512],
                                 optimize=True).astype(np.float32)
    return z8, zb, w8, wb, row512


def _host_assemble(outs, row512):
    # outs: 8 arrays (8, 128, 2*2*513) bf16; E/O halves per batch
    per_batch_E, per_batch_O = [], []
    for o in outs:
        o = np.asarray(o, dtype=np.float32)
        for b in range(B_PER_CORE):
            base = b * 2 * FRAMES
            per_batch_E.append(o[:, :, base:base + FRAMES])
            per_batch_O.append(
                o[:, :, base + FRAMES:base + 2 * FRAMES])
    E = np.stack(per_batch_E).reshape(BATCH, 2, 512, FRAMES)
    O = np.stack(per_batch_O).reshape(BATCH, 2, 512, FRAMES)

    outs_full = []
    for kern, msign in ((0, 1.0), (1, -1.0)):
        lo = E[:, kern] + O[:, kern]               # bins 0..511
        hi = E[:, kern] - O[:, kern]               # bins 1024-k
        if kern == 1:
            hi = -hi
        head = np.concatenate(
            [lo, row512[kern][:, None, :], hi[:, 511:0:-1], hi[:, 0:1]],
            axis=1)                                 # bins 0..1024
        full = np.concatenate([head, msign * head[:, 1023:0:-1]], axis=1)
        outs_full.append(np.ascontiguousarray(full, dtype=np.float32))
    return tuple(outs_full)


def kernel(x, wsin, wcos):
    from concourse.bass_utils import run_bass_kernel_spmd

    nc = _get_nc()
    z8, zb, w8, wb, row512 = _host_prep(x, wsin, wcos)
    in_maps = [
        {"z8": z8[i * B_PER_CORE:(i + 1) * B_PER_CORE],
         "zb": zb[i * B_PER_CORE:(i + 1) * B_PER_CORE],
         "w8": w8, "wb": wb}
        for i in range(CORES)
    ]
    res = run_bass_kernel_spmd(nc, in_maps, core_ids=list(range(CORES)))
    return _host_assemble(
        [res.results[i]["o"] for i in range(CORES)], row512)
